# revision 1
# baseline (speedup 1.0000x reference)
"""DAGNN forward on 8 Trainium2 NeuronCores.

Computation: a[:, :512] = x; for node i in topological (index) order:
a[:, i] = tanh(b[i] + sum_j W[i, j] * a[:, j]); y = a[:, 1536:2048].

Strategy:
- Data-parallel over batch: 8 cores x 256 rows each. Activations stored
  transposed on-chip: aT[node, batch] so matmuls contract over source
  nodes on the partition dim and outputs land directly in aT layout.
- Host computes DAG levels from the edge list, reorders nodes by level
  (stable), and packs dense 128x128 weight blocks:
    * off-diagonal blocks (prev 128-chunk -> dst 128-chunk): bulk PE work
      that overlaps with the critical path.
    * per-level-group "gather" blocks (in-chunk deps, columns masked to
      the group): one matmul per level group on the critical path.
- Kernel: for each of 12 computed chunks: accumulate off-diag matmuls
  into PSUM, then walk level groups: gather-matmul (in-chunk sources) ->
  ACT tanh (bias fused) writing the group's aT rows. Off-diag matmuls of
  the NEXT chunk are interleaved between groups to keep PE busy during
  ACT waits.
"""

import sys

for _p in ("/opt/trn_rl_repo",):
    if _p not in sys.path:
        sys.path.append(_p)

import numpy as np

N_NODES = 2048
N_IN = 512
N_OUT = 512
DEG = 32
BATCH = 2048
NCORES = 8
BCORE = BATCH // NCORES  # 256
CH = 128
NCOMP = (N_NODES - N_IN) // CH  # 12
NCH_IN = N_IN // CH  # 4


def _prep(edge_src, edge_dst, edge_w, b):
    """Level-sort the DAG and pack weight blocks.

    Returns a dict with everything the kernel builder + host assembly need.
    """
    edge_src = np.asarray(edge_src, dtype=np.int64)
    edge_dst = np.asarray(edge_dst, dtype=np.int64)
    edge_w = np.asarray(edge_w, dtype=np.float32)
    b = np.asarray(b, dtype=np.float32)

    src2 = edge_src.reshape(N_NODES - N_IN, DEG)
    level = np.zeros(N_NODES, np.int64)
    for i in range(N_IN, N_NODES):
        level[i] = level[src2[i - N_IN]].max() + 1

    comp = np.arange(N_IN, N_NODES)
    order = comp[np.argsort(level[N_IN:], kind="stable")]  # old ids, by level
    perm = np.concatenate([np.arange(N_IN), order])  # new -> old
    newpos = np.empty(N_NODES, np.int64)
    newpos[perm] = np.arange(N_NODES)

    # dense transposed weights in new coords: WT[src_new, dst_new]
    WT = np.zeros((N_NODES, N_NODES), np.float32)
    np.add.at(WT, (newpos[edge_src], newpos[edge_dst]), edge_w)

    lev_new = level[perm]
    bias128 = b[perm][N_IN:].reshape(NCOMP, CH).T.copy()  # [128, 12]

    # level groups per computed chunk
    chunk_groups = []
    for t in range(NCOMP):
        g0 = N_IN + t * CH
        levs = lev_new[g0 : g0 + CH]
        bnds = [0] + [i for i in range(1, CH) if levs[i] != levs[i - 1]] + [CH]
        chunk_groups.append(list(zip(bnds[:-1], bnds[1:])))

    # pack blocks into wflat [128, F]; per chunk: non-boundary offdiag,
    # boundary split into bd_pre (sources before the previous chunk's last
    # level group) + bd_last (last-group sources only), gather blocks.
    a32 = lambda v: (v // 32) * 32
    u32 = lambda v: -(-v // 32) * 32
    cols = []
    col = 0
    chunk_meta = []
    for t in range(NCOMP):
        dst0 = N_IN + t * CH
        nb = []  # (col_off, k) non-boundary offdiag
        bdpre = None  # (col_off, k) boundary block
        nsrc = NCH_IN + t  # source chunks 0..nsrc-1
        for k in range(nsrc):
            blk = WT[k * CH : (k + 1) * CH, dst0 : dst0 + CH]
            if not blk.any():
                continue
            if t >= 1 and k == nsrc - 1:
                cols.append(blk)
                bdpre = (col, k)
                col += CH
            else:
                cols.append(blk)
                nb.append((col, k))
                col += CH
        gth = []  # (col_off, r0, r1)
        D = WT[dst0 : dst0 + CH, dst0 : dst0 + CH]
        for gi, (r0, r1) in enumerate(chunk_groups[t]):
            if gi == 0:
                continue
            blk = np.zeros((CH, CH), np.float32)
            blk[:, r0:r1] = D[:, r0:r1]
            if not blk.any():
                gth.append(None)  # group has no in-chunk sources
                continue
            cols.append(blk)
            gth.append((col, r0, r1))
            col += CH
        chunk_meta.append(dict(nb=nb, bd=bdpre, gth=gth))

    wflat = np.ascontiguousarray(np.concatenate(cols, axis=1))  # [128, col]
    out_rows = newpos[np.arange(N_NODES - N_OUT, N_NODES)] - N_IN  # rows in outT

    return dict(
        perm=perm,
        newpos=newpos,
        level=level,
        chunk_groups=chunk_groups,
        chunk_meta=chunk_meta,
        wflat=wflat,
        bias128=bias128,
        out_rows=out_rows,
    )


def _emulate(prep, xT):
    """Numpy emulation of the exact block scheme (per core). xT: [512, B]."""
    B = xT.shape[1]
    wflat = prep["wflat"]
    aT = np.zeros((N_NODES, B), np.float32)
    aT[:N_IN] = xT
    bias = prep["bias128"]
    for t in range(NCOMP):
        meta = prep["chunk_meta"][t]
        dst0 = N_IN + t * CH
        psum = np.zeros((CH, B), np.float32)
        mms = list(meta["nb"])
        if meta["bd"]:
            mms.append(meta["bd"])
        for coloff, k in mms:
            blk = wflat[:, coloff : coloff + CH]
            psum += blk.T @ aT[k * CH : (k + 1) * CH]
        groups = prep["chunk_groups"][t]
        for gi, (r0, r1) in enumerate(groups):
            if gi > 0 and prep["chunk_meta"][t]["gth"][gi - 1] is not None:
                coloff, gr0, gr1 = prep["chunk_meta"][t]["gth"][gi - 1]
                assert (gr0, gr1) == (r0, r1)
                blk = wflat[:, coloff : coloff + CH]
                psum += blk.T @ aT[dst0 : dst0 + CH]
            aT[dst0 + r0 : dst0 + r1] = np.tanh(
                psum[r0:r1] + bias[r0:r1, t : t + 1]
            )
    return aT[N_IN:]  # [1536, B]


def _build_program(prep):
    """Build the Bass/Tile program (identical for all 8 cores)."""
    import concourse.bacc as bacc
    import concourse.tile as tile
    from concourse import mybir

    f32 = mybir.dt.float32
    f16 = mybir.dt.float16
    nc = bacc.Bacc(
        "TRN2",
        target_bir_lowering=False,
        debug=False,
        enable_asserts=False,
        num_devices=NCORES,
    )
    wflat = prep["wflat"]
    F = wflat.shape[1]

    xT_d = nc.dram_tensor("xT", [N_IN, BCORE], f16, kind="ExternalInput").ap()
    w_d = nc.dram_tensor("wflat", [CH, F], f16, kind="ExternalInput").ap()
    b_d = nc.dram_tensor("bias", [CH, NCOMP], f32, kind="ExternalInput").ap()
    out_d = nc.dram_tensor(
        "outT", [NCOMP * CH, BCORE], f16, kind="ExternalOutput"
    ).ap()

    meta = prep["chunk_meta"]
    groups = prep["chunk_groups"]

    # per-chunk W tile width (cols) and base offset within wflat
    chunk_w0 = []
    chunk_w1 = []
    for t in range(NCOMP):
        offs = [c for c, _ in meta[t]["nb"]]
        if meta[t]["bd"]:
            offs.append(meta[t]["bd"][0])
        offs += [g[0] for g in meta[t]["gth"] if g is not None]
        chunk_w0.append(min(offs))
        chunk_w1.append(max(offs) + CH)

    a32 = lambda v: (v // 32) * 32
    u32 = lambda v: -(-v // 32) * 32

    def m2_segs(w):
        # decompose [w, 128) into legal (base, size) output-partition windows
        if w == 32:
            return [(32, 32), (64, 64)]
        if w == 64:
            return [(64, 64)]
        if w == 96:
            return [(96, 32)]
        if w == 128:
            return []
        raise AssertionError(w)

    with tile.TileContext(nc) as tc:
        with (
            tc.tile_pool(name="aT", bufs=1) as aT_pool,
            tc.tile_pool(name="wpool", bufs=3) as w_pool,
            tc.tile_pool(name="small", bufs=1) as small_pool,
            tc.tile_pool(name="psum", bufs=3, space="PSUM") as psum_pool,
        ):
            # persistent activation tiles, one per 128-node chunk
            aT = [
                aT_pool.tile([CH, BCORE], f16, tag=f"aT{c}", name=f"aT{c}")
                for c in range(N_NODES // CH)
            ]
            bias_t = small_pool.tile([CH, NCOMP], f32, tag="bias")
            scratch = small_pool.tile([CH, 1], f32, tag="scratch")


            w_tiles = [None] * NCOMP

            def wdma(t):
                wid = chunk_w1[t] - chunk_w0[t]
                w_tiles[t] = w_pool.tile([CH, wid], f16, tag="w", name=f"w{t}")
                if t == 0:
                    # split so the first matmuls wait only on the off-diag
                    # blocks, not the whole span (gathers land in part B)
                    split = max(c for c, _ in meta[0]["nb"]) + CH - chunk_w0[0]
                    nc.sync.dma_start(
                        out=w_tiles[0][:, :split],
                        in_=w_d[:, chunk_w0[0] : chunk_w0[0] + split],
                    )
                    nc.sync.dma_start(
                        out=w_tiles[0][:, split:],
                        in_=w_d[:, chunk_w0[0] + split : chunk_w1[0]],
                    )
                    return
                nc.sync.dma_start(
                    out=w_tiles[t][:],
                    in_=w_d[:, chunk_w0[t] : chunk_w1[t]],
                )

            def wslice(t, coloff):
                a = coloff - chunk_w0[t]
                return w_tiles[t][:, a : a + CH]

            # per-chunk matmul bookkeeping for start/stop flags
            n_mms = []
            for t in range(NCOMP):
                n = len(meta[t]["nb"]) + (1 if meta[t]["bd"] else 0)
                n += sum(1 for g in meta[t]["gth"] if g is not None)
                n_mms.append(n)
            mm_count = [0] * NCOMP
            psum_t = [None] * NCOMP

            def emit_mm(t, coloff, rhs_tile, k1=CH, m0=0, m1=CH):
                if psum_t[t] is None:
                    psum_t[t] = psum_pool.tile(
                        [CH, BCORE], f32, tag="psum", name=f"psum{t}"
                    )
                first = mm_count[t] == 0
                last = mm_count[t] == n_mms[t] - 1
                a = coloff - chunk_w0[t]
                h = nc.tensor.matmul(
                    psum_t[t][m0:m1, :],
                    w_tiles[t][:k1, a + m0 : a + m1],
                    rhs_tile[:k1, :],
                    start=first,
                    stop=last,
                )
                mm_count[t] += 1
                return h

            # prologue: W0 config first so its transfer starts earliest,
            # then xT (needed by the same first matmuls), bias last
            wdma(0)
            for c in range(NCH_IN):
                nc.sync.dma_start(
                    out=aT[c][:], in_=xT_d[c * CH : (c + 1) * CH, :]
                )
            nc.sync.dma_start(out=bias_t[:], in_=b_d[:])
            # preload the tanh table set during the prologue
            nc.scalar.activation(
                scratch[:], bias_t[:, 0:1], mybir.ActivationFunctionType.Tanh
            )
            wdma(1)
            wdma(2)
            chain_mm = None
            for coloff, k in meta[0]["nb"]:
                chain_mm = emit_mm(0, coloff, aT[k])

            for t in range(NCOMP):
                if t + 3 < NCOMP:
                    wdma(t + 3)
                pending = list(meta[t + 1]["nb"]) if t + 1 < NCOMP else []
                gs = groups[t]
                n_slots = len(gs)
                fil_acc = 0
                for gi, (r0, r1) in enumerate(gs):
                    if gi > 0 and meta[t]["gth"][gi - 1] is not None:
                        coloff, _, _ = meta[t]["gth"][gi - 1]
                        chain_mm = emit_mm(t, coloff, aT[NCH_IN + t])
                    # Full-tile tanh: earlier groups' psum rows are final
                    # (idempotent rewrite); later groups' rows get fixed by
                    # their own ACT before any weighted read sees them.
                    nc.scalar.activation(
                        aT[NCH_IN + t][:, :],
                        psum_t[t][:, :],
                        mybir.ActivationFunctionType.Tanh,
                        bias=bias_t[:, t : t + 1],
                    )
                    # Bresenham spread: avoids runs of 2-filler slots that
                    # overflow the ACT-wait window and stretch the chain
                    fil_acc += len(meta[t + 1]["nb"]) if t + 1 < NCOMP else 0
                    take = fil_acc // n_slots
                    fil_acc -= take * n_slots
                    for _ in range(take):
                        if pending:
                            coloff, k = pending.pop(0)
                            emit_mm(t + 1, coloff, aT[k])
                while pending:
                    coloff, k = pending.pop(0)
                    emit_mm(t + 1, coloff, aT[k])
                if t + 1 < NCOMP and meta[t + 1]["bd"] is not None:
                    coloff, k = meta[t + 1]["bd"]
                    chain_mm = emit_mm(t + 1, coloff, aT[k])
                nc.sync.dma_start(
                    out=out_d[t * CH : (t + 1) * CH, :], in_=aT[NCH_IN + t][:]
                )

    nc.compile()
    return nc


def _make_in_maps(prep, x):
    x = np.asarray(x, dtype=np.float32)
    wflat = prep["wflat"]
    bias128 = prep["bias128"]
    in_maps = []
    for r in range(NCORES):
        xr = x[r * BCORE : (r + 1) * BCORE]  # [256, 512]
        in_maps.append(
            {
                "xT": np.ascontiguousarray(xr.T).astype(np.float16),
                "wflat": wflat.astype(np.float16),
                "bias": bias128,
            }
        )
    return in_maps


def _assemble(prep, results):
    out_rows = prep["out_rows"]
    y = np.empty((BATCH, N_OUT), np.float32)
    for r in range(NCORES):
        outT = results[r]["outT"].astype(np.float32)  # [1536, 256]
        y[r * BCORE : (r + 1) * BCORE, :] = outT[out_rows, :].T
    return y


def kernel(x, edge_w, b, edge_src, edge_dst, n_out, _trace=False):
    n_out = int(n_out)
    assert n_out == N_OUT, f"hardcoded for n_out={N_OUT}, got {n_out}"
    x = np.asarray(x, dtype=np.float32)
    assert x.shape == (BATCH, N_IN)

    from concourse.bass_utils import run_bass_kernel_spmd

    prep = _prep(edge_src, edge_dst, edge_w, b)
    nc = _build_program(prep)
    in_maps = _make_in_maps(prep, x)
    res = run_bass_kernel_spmd(
        nc, in_maps, core_ids=list(range(NCORES)), trace=_trace
    )
    y = _assemble(prep, res.results)
    if _trace:
        kernel._last_exec_time_ns = res.exec_time_ns
        kernel._last_results = res
    return y


if __name__ == "__main__":
    # quick host-side emulation check against a jax reference
    sys.path.insert(0, "/root/problem")
    import os

    os.environ.setdefault("JAX_PLATFORMS", "cpu")
    import reference

    inputs = {k: np.asarray(v) for k, v in reference.setup_inputs().items()}
    prep = _prep(
        inputs["edge_src"], inputs["edge_dst"], inputs["edge_w"], inputs["b"]
    )
    expected = np.asarray(
        reference.reference(**reference.setup_inputs())
    )
    xT = inputs["x"][:8].T.astype(np.float32)  # tiny batch slice
    aT = _emulate(prep, xT)
    got = aT[prep["out_rows"], :].T
    err = np.abs(got - expected[:8]).max()
    rel = err / np.abs(expected[:8]).max()
    print(f"emulation absmax err {err:.3e}  rel {rel:.3e}")



# revision 3
# speedup vs baseline: 1.1080x; 1.1080x over previous
"""DAGNN forward on 8 Trainium2 NeuronCores.

Computation: a[:, :512] = x; for node i in topological (index) order:
a[:, i] = tanh(b[i] + sum_j W[i, j] * a[:, j]); y = a[:, 1536:2048].

Strategy (v2):
- Data-parallel over batch: 8 cores x 256 rows each. Activations stored
  transposed on-chip: aT[node, batch].
- Host computes DAG levels, reorders nodes by level (stable), and packs
  nodes into LEVEL-ALIGNED chunks of <=128 nodes (no level spans a chunk
  boundary) -> exactly one tanh round per level (89 rounds), the serial
  floor for this DAG.
- Batch is split into S=2 halves with fully independent tiles; the two
  per-level chains interleave on PE/ACT so the scalar engine stays
  saturated: round time ~= 2 * tanh(128 free) instead of
  tanh(256) + matmul + sync latency.
- Per chunk: off-diagonal source blocks (prev chunks -> this chunk) are
  bulk matmuls scheduled as fillers during earlier rounds; the boundary
  block (prev chunk) and in-chunk per-level gather blocks ride the
  critical chain. Gather blocks are masked to the level's columns and
  contract only already-final rows.
"""

import sys

for _p in ("/opt/trn_rl_repo",):
    if _p not in sys.path:
        sys.path.append(_p)

import numpy as np

N_NODES = 2048
N_IN = 512
N_OUT = 512
DEG = 32
BATCH = 2048
NCORES = 8
BCORE = BATCH // NCORES  # 256
NCH_IN = N_IN // 128  # 4 input chunks of 128
NSPLIT = 2  # batch halves per core (independent interleaved chains)
HB = BCORE // NSPLIT  # 128


def _prep(edge_src, edge_dst, edge_w, b):
    """Level-sort the DAG, pack level-aligned chunks and weight blocks."""
    edge_src = np.asarray(edge_src, dtype=np.int64)
    edge_dst = np.asarray(edge_dst, dtype=np.int64)
    edge_w = np.asarray(edge_w, dtype=np.float32)
    b = np.asarray(b, dtype=np.float32)

    src2 = edge_src.reshape(N_NODES - N_IN, DEG)
    level = np.zeros(N_NODES, np.int64)
    for i in range(N_IN, N_NODES):
        level[i] = level[src2[i - N_IN]].max() + 1
    L = int(level.max())

    comp = np.arange(N_IN, N_NODES)
    order = comp[np.argsort(level[N_IN:], kind="stable")]  # old ids by level
    perm = np.concatenate([np.arange(N_IN), order])  # new -> old
    newpos = np.empty(N_NODES, np.int64)
    newpos[perm] = np.arange(N_NODES)

    # dense transposed weights in new coords: WT[src_new, dst_new]
    WT = np.zeros((N_NODES, N_NODES), np.float32)
    np.add.at(WT, (newpos[edge_src], newpos[edge_dst]), edge_w)

    lev_new = level[perm]  # sorted for computed region

    # level-aligned chunks: pack whole levels while size <= 128
    lev_sizes = [int((lev_new[N_IN:] == l).sum()) for l in range(1, L + 1)]
    chunks = []  # dict(base, sz, groups=[(r0, r1)])
    base = N_IN
    cur_sz = 0
    cur_groups = []
    for l, s in enumerate(lev_sizes, start=1):
        assert s > 0
        if cur_sz + s > 128:
            chunks.append(dict(base=base, sz=cur_sz, groups=cur_groups))
            base += cur_sz
            cur_sz = 0
            cur_groups = []
        cur_groups.append((cur_sz, cur_sz + s))
        cur_sz += s
    chunks.append(dict(base=base, sz=cur_sz, groups=cur_groups))
    C = len(chunks)

    # weight block packing into wflat [128, F]
    # per chunk t, in column order: offdiag (inputs then computed 0..t-2),
    # bd (t-1), gathers (levels 1..L_t-1)
    cols = []
    col = 0
    for t, ch in enumerate(chunks):
        d0, sz = ch["base"], ch["sz"]
        off = []  # (coloff, src_kind, src_idx, k_rows)
        for c in range(NCH_IN):
            blk = WT[c * 128 : (c + 1) * 128, d0 : d0 + sz]
            if blk.any():
                cols.append(blk)
                off.append((col, "in", c, 128))
                col += sz
        for s in range(max(0, t - 1)):
            sb, ssz = chunks[s]["base"], chunks[s]["sz"]
            blk = WT[sb : sb + ssz, d0 : d0 + sz]
            if blk.any():
                cols.append(blk)
                off.append((col, "ch", s, ssz))
                col += sz
        bd = None
        if t > 0:
            sb, ssz = chunks[t - 1]["base"], chunks[t - 1]["sz"]
            blk = WT[sb : sb + ssz, d0 : d0 + sz]
            if blk.any():
                cols.append(blk)
                bd = (col, ssz)
                col += sz
        gth = []  # per level index >=1: (coloff, k_rows) or None
        D = WT[d0 : d0 + sz, d0 : d0 + sz]
        for gi, (r0, r1) in enumerate(ch["groups"]):
            if gi == 0:
                continue
            blk = np.zeros((r0, sz), np.float32)
            blk[:, r0:r1] = D[:r0, r0:r1]
            if blk.any():
                cols.append(blk)
                gth.append((col, r0))
                col += sz
            else:
                gth.append(None)
        ch["off"] = off
        ch["bd"] = bd
        ch["gth"] = gth
        ch["w1"] = col
    F = col
    wflat = np.zeros((128, F), np.float32)
    c = 0
    for blk in cols:
        k, w = blk.shape
        wflat[:k, c : c + w] = blk
        c += w
    assert c == F

    # chunk col ranges
    w0 = 0
    for ch in chunks:
        ch["w0"] = w0
        w0 = ch["w1"]

    bias_pack = np.zeros((128, C), np.float32)
    for t, ch in enumerate(chunks):
        bias_pack[: ch["sz"], t] = b[perm][ch["base"] : ch["base"] + ch["sz"]]

    out_rows = newpos[np.arange(N_NODES - N_OUT, N_NODES)] - N_IN

    return dict(
        perm=perm,
        newpos=newpos,
        chunks=chunks,
        wflat=wflat,
        bias=bias_pack,
        out_rows=out_rows,
        n_rounds=sum(len(ch["groups"]) for ch in chunks),
    )


def _emulate(prep, xT):
    """Numpy emulation of the exact block scheme (per core). xT: [512, B]."""
    B = xT.shape[1]
    wflat = prep["wflat"]
    chunks = prep["chunks"]
    aT = np.zeros((N_NODES, B), np.float32)
    aT[:N_IN] = xT
    bias = prep["bias"]
    for t, ch in enumerate(chunks):
        d0, sz = ch["base"], ch["sz"]
        psum = np.zeros((sz, B), np.float32)
        for coloff, kind, s, k in ch["off"]:
            blk = wflat[:k, coloff : coloff + sz]
            if kind == "in":
                rows = aT[s * 128 : s * 128 + k]
            else:
                rows = aT[chunks[s]["base"] : chunks[s]["base"] + k]
            psum += blk.T @ rows
        if ch["bd"] is not None:
            coloff, k = ch["bd"]
            blk = wflat[:k, coloff : coloff + sz]
            sb = chunks[t - 1]["base"]
            psum += blk.T @ aT[sb : sb + k]
        for gi, (r0, r1) in enumerate(ch["groups"]):
            if gi > 0 and ch["gth"][gi - 1] is not None:
                coloff, k = ch["gth"][gi - 1]
                blk = wflat[:k, coloff : coloff + sz]
                psum += blk.T @ aT[d0 : d0 + k]
            # idempotent full-row tanh rewrite (matches kernel)
            aT[d0 : d0 + sz] = np.tanh(psum + bias[:sz, t : t + 1])
    return aT[N_IN:]  # [1536, B]


def _build_program(prep):
    """Build the Bass/Tile program (identical for all 8 cores)."""
    import concourse.bacc as bacc
    import concourse.tile as tile
    from concourse import mybir

    f32 = mybir.dt.float32
    f16 = mybir.dt.float16
    nc = bacc.Bacc(
        "TRN2",
        target_bir_lowering=False,
        debug=False,
        enable_asserts=False,
        num_devices=NCORES,
    )
    chunks = prep["chunks"]
    C = len(chunks)
    wflat = prep["wflat"]
    F = wflat.shape[1]
    NH = NSPLIT

    xT_d = nc.dram_tensor("xT", [128, NCH_IN * BCORE], f16, kind="ExternalInput").ap()
    w_d = nc.dram_tensor("wflat", [128, F], f16, kind="ExternalInput").ap()
    b_d = nc.dram_tensor("bias", [128, C], f32, kind="ExternalInput").ap()
    out_d = nc.dram_tensor(
        "outT", [N_NODES - N_IN, BCORE], f16, kind="ExternalOutput"
    ).ap()

    with tile.TileContext(nc) as tc:
        with (
            tc.tile_pool(name="aT", bufs=1) as aT_pool,
            tc.tile_pool(name="wpool", bufs=4) as w_pool,
            tc.tile_pool(name="small", bufs=1) as small_pool,
            tc.tile_pool(name="psum", bufs=3 * NH, space="PSUM") as psum_pool,
        ):
            # persistent tiles
            xin = aT_pool.tile([128, NCH_IN * BCORE], f16, tag="xin", name="xin")
            aT = [
                [
                    aT_pool.tile([128, HB], f16, tag=f"aT{t}h{h}", name=f"aT{t}h{h}")
                    for h in range(NH)
                ]
                for t in range(C)
            ]
            bias_t = small_pool.tile([128, C], f32, tag="bias")
            scratch = small_pool.tile([128, 1], f32, tag="scratch")

            def xin_view(c, h):
                return xin[:, c * BCORE + h * HB : c * BCORE + (h + 1) * HB]

            w_tiles = [None] * C

            def wdma(t, split_off=False):
                ch = chunks[t]
                wid = ch["w1"] - ch["w0"]
                w_tiles[t] = w_pool.tile([128, wid], f16, tag="w", name=f"w{t}")
                if split_off:
                    # first transfer covers only the off-diag blocks so the
                    # prologue matmuls don't wait for gathers
                    split = (ch["bd"][0] if ch["bd"] else ch["w1"]) - ch["w0"]
                    if ch["off"]:
                        split = max(split, ch["off"][-1][0] + ch["sz"] - ch["w0"])
                    nc.sync.dma_start(
                        out=w_tiles[t][:, :split],
                        in_=w_d[:, ch["w0"] : ch["w0"] + split],
                    )
                    if split < wid:
                        nc.sync.dma_start(
                            out=w_tiles[t][:, split:],
                            in_=w_d[:, ch["w0"] + split : ch["w1"]],
                        )
                    return
                nc.sync.dma_start(
                    out=w_tiles[t][:], in_=w_d[:, ch["w0"] : ch["w1"]]
                )

            def wslice(t, coloff, k):
                a = coloff - chunks[t]["w0"]
                return w_tiles[t][:k, a : a + chunks[t]["sz"]]

            # psum bookkeeping
            psum_t = [[None] * NH for _ in range(C)]
            started = [[False] * NH for _ in range(C)]
            n_mms = []
            for t, ch in enumerate(chunks):
                n = len(ch["off"]) + (1 if ch["bd"] else 0)
                n += sum(1 for g in ch["gth"] if g is not None)
                n_mms.append(n)
            mm_count = [[0] * NH for _ in range(C)]

            def emit_mm(t, coloff, k, rhs_tile, h):
                ch = chunks[t]
                if psum_t[t][h] is None:
                    psum_t[t][h] = psum_pool.tile(
                        [128, HB], f32, tag="psum", name=f"ps{t}h{h}"
                    )
                first = mm_count[t][h] == 0
                last = mm_count[t][h] == n_mms[t] - 1
                nc.tensor.matmul(
                    psum_t[t][h][: ch["sz"], :],
                    wslice(t, coloff, k),
                    rhs_tile[:k, :],
                    start=first,
                    stop=last,
                )
                mm_count[t][h] += 1

            # filler queue: (dst_t, h, coloff, k, src_kind, src_idx)
            fq = []
            enq_done = set()

            def enqueue_fillers(t):
                # called at start of chunk t: dsts t+1, t+2 with ready srcs
                for d in (t + 1, t + 2):
                    if d >= C:
                        continue
                    for coloff, kind, s, k in chunks[d]["off"]:
                        if kind == "ch" and s > t - 1:
                            continue
                        key = (d, coloff)
                        if key in enq_done:
                            continue
                        enq_done.add(key)
                        for h in range(NH):
                            fq.append((d, h, coloff, k, kind, s))

            def pop_fillers(n, dst_max=None):
                done = 0
                while fq and done < n:
                    if dst_max is not None and fq[0][0] > dst_max:
                        break
                    d, h, coloff, k, kind, s = fq.pop(0)
                    src = xin_view(s, h) if kind == "in" else aT[s][h]
                    emit_mm(d, coloff, k, src, h)
                    done += 1
                return done

            # ---- prologue ----
            # tanh table preload on garbage scratch (no DMA dependency)
            nc.scalar.activation(
                scratch[:], scratch[:], mybir.ActivationFunctionType.Tanh
            )
            wdma(0, split_off=True)
            nc.sync.dma_start(out=xin[:], in_=xT_d[:])
            nc.sync.dma_start(out=bias_t[:], in_=b_d[:])
            wdma(1)
            wdma(2)

            # chunk 0 off-diag (inputs) for both halves
            for h in range(NH):
                for coloff, kind, s, k in chunks[0]["off"]:
                    emit_mm(0, coloff, k, xin_view(s, h), h)

            # ---- rounds ----
            for t, ch in enumerate(chunks):
                if t + 3 < C:
                    wdma(t + 3)
                enqueue_fillers(t)
                sz = ch["sz"]
                ngroups = len(ch["groups"])
                for gi, (r0, r1) in enumerate(ch["groups"]):
                    last_round = gi == ngroups - 1
                    for h in range(NH):
                        if gi == 0:
                            if ch["bd"] is not None:
                                coloff, k = ch["bd"]
                                emit_mm(t, coloff, k, aT[t - 1][h], h)
                        else:
                            g = ch["gth"][gi - 1]
                            if g is not None:
                                coloff, k = g
                                emit_mm(t, coloff, k, aT[t][h], h)
                        # full-row idempotent tanh (rows of later levels get
                        # garbage, rewritten by their own round)
                        nc.scalar.activation(
                            aT[t][h][:sz, :],
                            psum_t[t][h][:sz, :],
                            mybir.ActivationFunctionType.Tanh,
                            bias=bias_t[:sz, t : t + 1],
                        )
                        pop_fillers(1 if h == 0 else 3)
                # drain fillers targeting chunk t+1 before its rounds start
                pop_fillers(len(fq), dst_max=t + 1)
                # output DMA (issued from the idle gpsimd engine so the
                # sync queue stays free for weight streaming)
                g0 = ch["base"] - N_IN
                for h in range(NH):
                    nc.gpsimd.dma_start(
                        out=out_d[g0 : g0 + sz, h * HB : (h + 1) * HB],
                        in_=aT[t][h][:sz, :],
                    )

    nc.compile()
    return nc


def _make_in_maps(prep, x):
    x = np.asarray(x, dtype=np.float32)
    wflat16 = prep["wflat"].astype(np.float16)
    bias = prep["bias"]
    in_maps = []
    for r in range(NCORES):
        xr = x[r * BCORE : (r + 1) * BCORE]  # [256, 512]
        xT2 = (
            xr.T.reshape(NCH_IN, 128, BCORE)
            .transpose(1, 0, 2)
            .reshape(128, NCH_IN * BCORE)
        )
        in_maps.append(
            {
                "xT": np.ascontiguousarray(xT2).astype(np.float16),
                "wflat": wflat16,
                "bias": bias,
            }
        )
    return in_maps


def _assemble(prep, results):
    out_rows = prep["out_rows"]
    y = np.empty((BATCH, N_OUT), np.float32)
    for r in range(NCORES):
        outT = results[r]["outT"].astype(np.float32)  # [1536, 256]
        y[r * BCORE : (r + 1) * BCORE, :] = outT[out_rows, :].T
    return y


def kernel(x, edge_w, b, edge_src, edge_dst, n_out, _trace=False):
    n_out = int(n_out)
    assert n_out == N_OUT, f"hardcoded for n_out={N_OUT}, got {n_out}"
    x = np.asarray(x, dtype=np.float32)
    assert x.shape == (BATCH, N_IN)

    from concourse.bass_utils import run_bass_kernel_spmd

    prep = _prep(edge_src, edge_dst, edge_w, b)
    nc = _build_program(prep)
    in_maps = _make_in_maps(prep, x)
    res = run_bass_kernel_spmd(
        nc, in_maps, core_ids=list(range(NCORES)), trace=_trace
    )
    y = _assemble(prep, res.results)
    if _trace:
        kernel._last_exec_time_ns = res.exec_time_ns
        kernel._last_results = res
    return y


if __name__ == "__main__":
    # host-side emulation check against the jax reference
    sys.path.insert(0, "/root/problem")
    import os

    os.environ.setdefault("JAX_PLATFORMS", "cpu")

    inputs = {
        k: np.load(f"/tmp/ref_{k}.npy")
        for k in ("x", "edge_w", "b", "edge_src", "edge_dst")
    }
    expected = np.load("/tmp/ref_out.npy")
    prep = _prep(
        inputs["edge_src"], inputs["edge_dst"], inputs["edge_w"], inputs["b"]
    )
    print(
        f"chunks={len(prep['chunks'])} rounds={prep['n_rounds']} "
        f"F={prep['wflat'].shape[1]}"
    )
    xT = inputs["x"][:8].T.astype(np.float32)  # tiny batch slice
    aT = _emulate(prep, xT)
    got = aT[prep["out_rows"], :].T
    err = np.abs(got - expected[:8]).max()
    rel = err / np.abs(expected[:8]).max()
    print(f"emulation absmax err {err:.3e}  rel {rel:.3e}")


# revision 9
# speedup vs baseline: 1.1485x; 1.0365x over previous
"""DAGNN forward on 8 Trainium2 NeuronCores.

Computation: a[:, :512] = x; for node i in topological (index) order:
a[:, i] = tanh(b[i] + sum_j W[i, j] * a[:, j]); y = a[:, 1536:2048].

Strategy (v2):
- Data-parallel over batch: 8 cores x 256 rows each. Activations stored
  transposed on-chip: aT[node, batch].
- Host computes DAG levels, reorders nodes by level (stable), and packs
  nodes into LEVEL-ALIGNED chunks of <=128 nodes (no level spans a chunk
  boundary) -> exactly one tanh round per level (89 rounds), the serial
  floor for this DAG.
- Batch is split into S=2 halves with fully independent tiles; the two
  per-level chains interleave on PE/ACT so the scalar engine stays
  saturated: round time ~= 2 * tanh(128 free) instead of
  tanh(256) + matmul + sync latency.
- Per chunk: off-diagonal source blocks (prev chunks -> this chunk) are
  bulk matmuls scheduled as fillers during earlier rounds; the boundary
  block (prev chunk) and in-chunk per-level gather blocks ride the
  critical chain. Gather blocks are masked to the level's columns and
  contract only already-final rows.
"""

import sys

for _p in ("/opt/trn_rl_repo",):
    if _p not in sys.path:
        sys.path.append(_p)

import numpy as np

N_NODES = 2048
N_IN = 512
N_OUT = 512
DEG = 32
BATCH = 2048
NCORES = 8
BCORE = BATCH // NCORES  # 256
NCH_IN = N_IN // 128  # 4 input chunks of 128
NSPLIT = 2  # batch halves per core (independent interleaved chains)
HB = BCORE // NSPLIT  # 128
DUMF = 128  # free size of PE-warming dummy matmuls
DUM_PRE = 30  # prologue dummies (ramp PE to full p-state before round 1)
DUM_HALF = 4  # dummies per half-round (keep PE busy through tanh windows)


def _prep(edge_src, edge_dst, edge_w, b):
    """Level-sort the DAG, pack level-aligned chunks and weight blocks."""
    edge_src = np.asarray(edge_src, dtype=np.int64)
    edge_dst = np.asarray(edge_dst, dtype=np.int64)
    edge_w = np.asarray(edge_w, dtype=np.float32)
    b = np.asarray(b, dtype=np.float32)

    src2 = edge_src.reshape(N_NODES - N_IN, DEG)
    level = np.zeros(N_NODES, np.int64)
    for i in range(N_IN, N_NODES):
        level[i] = level[src2[i - N_IN]].max() + 1
    L = int(level.max())

    comp = np.arange(N_IN, N_NODES)
    order = comp[np.argsort(level[N_IN:], kind="stable")]  # old ids by level
    perm = np.concatenate([np.arange(N_IN), order])  # new -> old
    newpos = np.empty(N_NODES, np.int64)
    newpos[perm] = np.arange(N_NODES)

    # dense transposed weights in new coords: WT[src_new, dst_new]
    WT = np.zeros((N_NODES, N_NODES), np.float32)
    np.add.at(WT, (newpos[edge_src], newpos[edge_dst]), edge_w)

    lev_new = level[perm]  # sorted for computed region

    # level-aligned chunks: pack whole levels while size <= 128
    lev_sizes = [int((lev_new[N_IN:] == l).sum()) for l in range(1, L + 1)]
    chunks = []  # dict(base, sz, groups=[(r0, r1)])
    base = N_IN
    cur_sz = 0
    cur_groups = []
    for l, s in enumerate(lev_sizes, start=1):
        assert s > 0
        if cur_sz + s > 128:
            chunks.append(dict(base=base, sz=cur_sz, groups=cur_groups))
            base += cur_sz
            cur_sz = 0
            cur_groups = []
        cur_groups.append((cur_sz, cur_sz + s))
        cur_sz += s
    chunks.append(dict(base=base, sz=cur_sz, groups=cur_groups))
    C = len(chunks)

    # weight block packing into wflat [128, F]
    # per chunk t, in column order: offdiag (inputs then computed 0..t-2),
    # bd (t-1), gathers (levels 1..L_t-1)
    cols = []
    col = 0
    for t, ch in enumerate(chunks):
        d0, sz = ch["base"], ch["sz"]
        off = []  # (coloff, src_kind, src_idx, k_rows)
        for c in range(NCH_IN):
            blk = WT[c * 128 : (c + 1) * 128, d0 : d0 + sz]
            if blk.any():
                cols.append(blk)
                off.append((col, "in", c, 128))
                col += sz
        for s in range(max(0, t - 1)):
            sb, ssz = chunks[s]["base"], chunks[s]["sz"]
            blk = WT[sb : sb + ssz, d0 : d0 + sz]
            if blk.any():
                cols.append(blk)
                off.append((col, "ch", s, ssz))
                col += sz
        bd = None
        if t > 0:
            sb, ssz = chunks[t - 1]["base"], chunks[t - 1]["sz"]
            blk = WT[sb : sb + ssz, d0 : d0 + sz]
            if blk.any():
                cols.append(blk)
                bd = (col, ssz)
                col += sz
        gth = []  # per level index >=1: (coloff, k_rows) or None
        D = WT[d0 : d0 + sz, d0 : d0 + sz]
        for gi, (r0, r1) in enumerate(ch["groups"]):
            if gi == 0:
                continue
            blk = np.zeros((r0, sz), np.float32)
            blk[:, r0:r1] = D[:r0, r0:r1]
            if blk.any():
                cols.append(blk)
                gth.append((col, r0))
                col += sz
            else:
                gth.append(None)
        ch["off"] = off
        ch["bd"] = bd
        ch["gth"] = gth
        ch["w1"] = col
    F = col
    wflat = np.zeros((128, F), np.float32)
    c = 0
    for blk in cols:
        k, w = blk.shape
        wflat[:k, c : c + w] = blk
        c += w
    assert c == F

    # chunk col ranges
    w0 = 0
    for ch in chunks:
        ch["w0"] = w0
        w0 = ch["w1"]

    bias_pack = np.zeros((128, C), np.float32)
    for t, ch in enumerate(chunks):
        bias_pack[: ch["sz"], t] = b[perm][ch["base"] : ch["base"] + ch["sz"]]

    out_rows = newpos[np.arange(N_NODES - N_OUT, N_NODES)] - N_IN

    return dict(
        perm=perm,
        newpos=newpos,
        chunks=chunks,
        wflat=wflat,
        bias=bias_pack,
        out_rows=out_rows,
        n_rounds=sum(len(ch["groups"]) for ch in chunks),
    )


def _emulate(prep, xT):
    """Numpy emulation of the exact block scheme (per core). xT: [512, B]."""
    B = xT.shape[1]
    wflat = prep["wflat"]
    chunks = prep["chunks"]
    aT = np.zeros((N_NODES, B), np.float32)
    aT[:N_IN] = xT
    bias = prep["bias"]
    for t, ch in enumerate(chunks):
        d0, sz = ch["base"], ch["sz"]
        psum = np.zeros((sz, B), np.float32)
        for coloff, kind, s, k in ch["off"]:
            blk = wflat[:k, coloff : coloff + sz]
            if kind == "in":
                rows = aT[s * 128 : s * 128 + k]
            else:
                rows = aT[chunks[s]["base"] : chunks[s]["base"] + k]
            psum += blk.T @ rows
        if ch["bd"] is not None:
            coloff, k = ch["bd"]
            blk = wflat[:k, coloff : coloff + sz]
            sb = chunks[t - 1]["base"]
            psum += blk.T @ aT[sb : sb + k]
        for gi, (r0, r1) in enumerate(ch["groups"]):
            if gi > 0 and ch["gth"][gi - 1] is not None:
                coloff, k = ch["gth"][gi - 1]
                blk = wflat[:k, coloff : coloff + sz]
                psum += blk.T @ aT[d0 : d0 + k]
            # idempotent full-row tanh rewrite (matches kernel)
            aT[d0 : d0 + sz] = np.tanh(psum + bias[:sz, t : t + 1])
    return aT[N_IN:]  # [1536, B]


def _build_program(prep):
    """Build the Bass/Tile program (identical for all 8 cores)."""
    import concourse.bacc as bacc
    import concourse.tile as tile
    from concourse import mybir

    f32 = mybir.dt.float32
    f16 = mybir.dt.float16
    nc = bacc.Bacc(
        "TRN2",
        target_bir_lowering=False,
        debug=False,
        enable_asserts=False,
        num_devices=NCORES,
    )
    chunks = prep["chunks"]
    C = len(chunks)
    wflat = prep["wflat"]
    F = wflat.shape[1]
    NH = NSPLIT

    xT_d = nc.dram_tensor("xT", [128, NCH_IN * BCORE], f16, kind="ExternalInput").ap()
    w_d = nc.dram_tensor("wflat", [128, F], f16, kind="ExternalInput").ap()
    b_d = nc.dram_tensor("bias", [128, C], f32, kind="ExternalInput").ap()
    out_d = nc.dram_tensor(
        "outT", [N_NODES - N_IN, BCORE], f16, kind="ExternalOutput"
    ).ap()

    with tile.TileContext(nc) as tc:
        with (
            tc.tile_pool(name="aT", bufs=1) as aT_pool,
            tc.tile_pool(name="wpool", bufs=4) as w_pool,
            tc.tile_pool(name="small", bufs=1) as small_pool,
            tc.tile_pool(name="psum", bufs=3 * NH, space="PSUM") as psum_pool,
            tc.tile_pool(name="dpsum", bufs=1, space="PSUM") as dpsum_pool,
        ):
            # persistent tiles
            xin = aT_pool.tile([128, NCH_IN * BCORE], f16, tag="xin", name="xin")
            aT = [
                [
                    aT_pool.tile([128, HB], f16, tag=f"aT{t}h{h}", name=f"aT{t}h{h}")
                    for h in range(NH)
                ]
                for t in range(C)
            ]
            bias_t = small_pool.tile([128, C], f32, tag="bias")
            scratch = small_pool.tile([128, 1], f32, tag="scratch")
            # dummy matmul operand + sink: a stream of dummy matmuls keeps
            # the PE pipeline continuously busy, which (a) ramps the tensor
            # engine to its full p-state (2.4 GHz after 3us of continuous
            # execution; idle gaps reset it to 0.65 GHz) and (b) hides the
            # SBUF pipeline-fill latency of the critical chain matmuls.
            dummyw = small_pool.tile([128, 128], f16, tag="dummyw")
            dummy_ps = dpsum_pool.tile([128, DUMF], f32, tag="dps")

            def emit_dummy(n):
                for _ in range(n):
                    nc.tensor.matmul(
                        dummy_ps[:, :],
                        dummyw[:, :],
                        dummyw[:, :DUMF],
                        start=True,
                        stop=True,
                    )

            def xin_view(c, h):
                return xin[:, c * BCORE + h * HB : c * BCORE + (h + 1) * HB]

            w_tiles = [None] * C

            def wdma(t, split_off=False):
                ch = chunks[t]
                wid = ch["w1"] - ch["w0"]
                w_tiles[t] = w_pool.tile([128, wid], f16, tag="w", name=f"w{t}")
                if split_off:
                    # first transfer covers only the off-diag blocks so the
                    # prologue matmuls don't wait for gathers
                    split = (ch["bd"][0] if ch["bd"] else ch["w1"]) - ch["w0"]
                    if ch["off"]:
                        split = max(split, ch["off"][-1][0] + ch["sz"] - ch["w0"])
                    nc.sync.dma_start(
                        out=w_tiles[t][:, :split],
                        in_=w_d[:, ch["w0"] : ch["w0"] + split],
                    )
                    if split < wid:
                        nc.sync.dma_start(
                            out=w_tiles[t][:, split:],
                            in_=w_d[:, ch["w0"] + split : ch["w1"]],
                        )
                    return
                nc.sync.dma_start(
                    out=w_tiles[t][:], in_=w_d[:, ch["w0"] : ch["w1"]]
                )

            def wslice(t, coloff, k):
                a = coloff - chunks[t]["w0"]
                return w_tiles[t][:k, a : a + chunks[t]["sz"]]

            # psum bookkeeping
            psum_t = [[None] * NH for _ in range(C)]
            started = [[False] * NH for _ in range(C)]
            n_mms = []
            for t, ch in enumerate(chunks):
                n = len(ch["off"]) + (1 if ch["bd"] else 0)
                n += sum(1 for g in ch["gth"] if g is not None)
                n_mms.append(n)
            mm_count = [[0] * NH for _ in range(C)]

            def emit_mm(t, coloff, k, rhs_tile, h):
                ch = chunks[t]
                if psum_t[t][h] is None:
                    psum_t[t][h] = psum_pool.tile(
                        [128, HB], f32, tag="psum", name=f"ps{t}h{h}"
                    )
                first = mm_count[t][h] == 0
                last = mm_count[t][h] == n_mms[t] - 1
                nc.tensor.matmul(
                    psum_t[t][h][: ch["sz"], :],
                    wslice(t, coloff, k),
                    rhs_tile[:k, :],
                    start=first,
                    stop=last,
                )
                mm_count[t][h] += 1

            # filler queue: (dst_t, h, coloff, k, src_kind, src_idx)
            fq = []
            enq_done = set()

            def enqueue_fillers(t):
                # called at start of chunk t: dsts t+1, t+2 with ready srcs
                for d in (t + 1, t + 2):
                    if d >= C:
                        continue
                    for coloff, kind, s, k in chunks[d]["off"]:
                        if kind == "ch" and s > t - 1:
                            continue
                        key = (d, coloff)
                        if key in enq_done:
                            continue
                        enq_done.add(key)
                        for h in range(NH):
                            fq.append((d, h, coloff, k, kind, s))

            def pop_fillers(n, dst_max=None):
                done = 0
                while fq and done < n:
                    if dst_max is not None and fq[0][0] > dst_max:
                        break
                    d, h, coloff, k, kind, s = fq.pop(0)
                    src = xin_view(s, h) if kind == "in" else aT[s][h]
                    emit_mm(d, coloff, k, src, h)
                    done += 1
                return done

            # ---- prologue ----
            # tanh table preload on garbage scratch (no DMA dependency)
            nc.scalar.activation(
                scratch[:], scratch[:], mybir.ActivationFunctionType.Tanh
            )
            nc.vector.memset(dummyw[:], 0.0)
            wdma(0, split_off=True)
            nc.sync.dma_start(out=xin[:], in_=xT_d[:])
            nc.sync.dma_start(out=bias_t[:], in_=b_d[:])
            wdma(1)
            wdma(2)
            # start the PE ramp while the prologue DMAs stream
            emit_dummy(DUM_PRE)

            # chunk 0 off-diag (inputs) for both halves
            for h in range(NH):
                for coloff, kind, s, k in chunks[0]["off"]:
                    emit_mm(0, coloff, k, xin_view(s, h), h)

            # ---- rounds ----
            for t, ch in enumerate(chunks):
                if t + 3 < C:
                    wdma(t + 3)
                enqueue_fillers(t)
                sz = ch["sz"]
                ngroups = len(ch["groups"])
                for gi, (r0, r1) in enumerate(ch["groups"]):
                    last_round = gi == ngroups - 1
                    for h in range(NH):
                        if gi == 0:
                            if ch["bd"] is not None:
                                coloff, k = ch["bd"]
                                emit_mm(t, coloff, k, aT[t - 1][h], h)
                        else:
                            g = ch["gth"][gi - 1]
                            if g is not None:
                                coloff, k = g
                                emit_mm(t, coloff, k, aT[t][h], h)
                        # full-row idempotent tanh (rows of later levels get
                        # garbage, rewritten by their own round)
                        nc.scalar.activation(
                            aT[t][h][:sz, :],
                            psum_t[t][h][:sz, :],
                            mybir.ActivationFunctionType.Tanh,
                            bias=bias_t[:sz, t : t + 1],
                        )
                        n_fill = pop_fillers(1 if h == 0 else 3)
                        emit_dummy(max(0, DUM_HALF - n_fill))
                # drain fillers targeting chunk t+1 before its rounds start
                pop_fillers(len(fq), dst_max=t + 1)
                # output DMA (issued from the idle gpsimd engine so the
                # sync queue stays free for weight streaming)
                g0 = ch["base"] - N_IN
                for h in range(NH):
                    nc.gpsimd.dma_start(
                        out=out_d[g0 : g0 + sz, h * HB : (h + 1) * HB],
                        in_=aT[t][h][:sz, :],
                    )

    nc.compile()
    return nc


def _make_in_maps(prep, x):
    x = np.asarray(x, dtype=np.float32)
    wflat16 = prep["wflat"].astype(np.float16)
    bias = prep["bias"]
    in_maps = []
    for r in range(NCORES):
        xr = x[r * BCORE : (r + 1) * BCORE]  # [256, 512]
        xT2 = (
            xr.T.reshape(NCH_IN, 128, BCORE)
            .transpose(1, 0, 2)
            .reshape(128, NCH_IN * BCORE)
        )
        in_maps.append(
            {
                "xT": np.ascontiguousarray(xT2).astype(np.float16),
                "wflat": wflat16,
                "bias": bias,
            }
        )
    return in_maps


def _assemble(prep, results):
    out_rows = prep["out_rows"]
    y = np.empty((BATCH, N_OUT), np.float32)
    for r in range(NCORES):
        outT = results[r]["outT"].astype(np.float32)  # [1536, 256]
        y[r * BCORE : (r + 1) * BCORE, :] = outT[out_rows, :].T
    return y


def kernel(x, edge_w, b, edge_src, edge_dst, n_out, _trace=False):
    n_out = int(n_out)
    assert n_out == N_OUT, f"hardcoded for n_out={N_OUT}, got {n_out}"
    x = np.asarray(x, dtype=np.float32)
    assert x.shape == (BATCH, N_IN)

    from concourse.bass_utils import run_bass_kernel_spmd

    prep = _prep(edge_src, edge_dst, edge_w, b)
    nc = _build_program(prep)
    in_maps = _make_in_maps(prep, x)
    res = run_bass_kernel_spmd(
        nc, in_maps, core_ids=list(range(NCORES)), trace=_trace
    )
    y = _assemble(prep, res.results)
    if _trace:
        kernel._last_exec_time_ns = res.exec_time_ns
        kernel._last_results = res
    return y


if __name__ == "__main__":
    # host-side emulation check against the jax reference
    sys.path.insert(0, "/root/problem")
    import os

    os.environ.setdefault("JAX_PLATFORMS", "cpu")

    inputs = {
        k: np.load(f"/tmp/ref_{k}.npy")
        for k in ("x", "edge_w", "b", "edge_src", "edge_dst")
    }
    expected = np.load("/tmp/ref_out.npy")
    prep = _prep(
        inputs["edge_src"], inputs["edge_dst"], inputs["edge_w"], inputs["b"]
    )
    print(
        f"chunks={len(prep['chunks'])} rounds={prep['n_rounds']} "
        f"F={prep['wflat'].shape[1]}"
    )
    xT = inputs["x"][:8].T.astype(np.float32)  # tiny batch slice
    aT = _emulate(prep, xT)
    got = aT[prep["out_rows"], :].T
    err = np.abs(got - expected[:8]).max()
    rel = err / np.abs(expected[:8]).max()
    print(f"emulation absmax err {err:.3e}  rel {rel:.3e}")
